# revision 42
# baseline (speedup 1.0000x reference)
"""Trainium2 8-core GQA attention kernel (tensor-parallel over heads).

Strategy (8 NeuronCores, SPMD):
  - Core c owns q-heads [4c..4c+4) and kv-head c (GQA groups stay aligned).
  - Software-pipelined emission: attention for chunk c_i is emitted finely
    INTERLEAVED with the projection matmuls of chunk c_{i+1} (and the last
    chunk with phase C's matmuls), so the in-order PE queue always has
    independent work during attention's exp-latency bubbles. The final
    chunk's attention is split by heads: heads 0-1 ride as extra filler at
    the end of the last projection slot, heads 2-3 interleave with phase C,
    so the final AllGather -> wo-matmul chain starts as early as possible.
  - qkvT = wqkv_c^T @ x^T computed feature-major (K/V column group first);
    Q^T lives in a transient 2-deep ring (only its own chunk's attention
    needs it), K^T is persistent [hd, TOK]; V^T is transposed per-chunk
    (behind the next matmul group, so the PE never waits on the V copy)
    into persistent V[k,d] tiles.
  - Scores computed transposed (S^T[k,q]) so exp(S^T) feeds the PV matmul
    (lhsT = V[k,d]) with zero P transposes; fully-masked causal blocks are
    skipped; diagonal blocks are N-restricted to the valid q-range and get a
    compact [k,128] multiplicative mask on the triangle subblock only.
  - Denominators: exp blocks accumulated per head on DVE, one indicator-
    column matmul per (chunk, head) landing head h's sums on the 32-aligned
    PSUM row 32h; the tail does a per-head approx reciprocal (partition-0
    copy first: custom DVE ops ignore partition offsets) + gpsimd
    partition_broadcast (no PE broadcast matmuls), normalize, bounce DMA +
    AllGather, all deferred into the next chunk's interleave slot.
  - PSUM: 3 stacked pools -- attention st/outT/den (5 banks) for the whole
    kernel, projection accumulators (3 banks) during phase A, phase C
    accumulators (3 banks) reusing A's banks after it closes.
  - DMA: dma_start costs ~0.85us of issue time on the issuing queue, so
    startup weights/x loads are batched 4 k-tiles per DMA and spread across
    the sync/scalar/gpsimd queues in consumption order; wo and the masks
    preload on the gpsimd queue during phase A.
All PE math in bf16 (f32 PSUM accumulation).
"""

import numpy as np
import ml_dtypes

import concourse.bass as bass
import concourse.mybir as mybir
import concourse.tile as tile
from concourse import bacc
from concourse.bass_utils import run_bass_kernel_spmd

BF16 = mybir.dt.bfloat16
F32 = mybir.dt.float32
HD = 128            # head dim
HHD = HD // 2       # rope half
P = 128             # partitions
QCH = 512           # q-chunk / token-chunk size
KT = 128            # k tile (partition dim)
SCALE = 1.0 / np.sqrt(HD)
HEADSPLIT = True
RESTRICT = True     # N-restrict diagonal score/PV matmuls to valid q range


def build_graph(NB, S, D, HPC, NCORES, block_cls, n_mixed, qc_mask):
    """Build the per-core SPMD graph.

    block_cls[(qc, kt)] -> 'full' | 'skip' | int (mixed-mask slot index)
    qc_mask[qc] -> (first_slot, count) of that q-chunk's mixed-mask slots
    """
    TOK = NB * S
    QF = HPC * HD           # q features per core
    FLOC = QF + 2 * HD      # local qkv features (q + k + v)
    MT = FLOC // P          # feature tiles (q tiles + 1 k + 1 v)
    KD = D // P             # contraction tiles over model dim
    KH = KD // 2
    NQC = S // QCH          # q chunks per batch
    NKT = S // KT           # k tiles per batch
    KTC = QCH // KT         # k tiles per token chunk
    ODPC = D // NCORES      # output dims per core
    NCHK = TOK // QCH       # token chunks overall
    n_mask = max(n_mixed, 1)

    nc = bacc.Bacc("TRN2", target_bir_lowering=False, debug=False,
                   num_devices=NCORES)

    # pre-swizzled on host: per-partition-contiguous slabs
    # xt_d[p, ci*KD*QCH + ko*QCH + t] = x[ci*QCH+t, ko*P+p]
    # wqkv_d[p, (gi*KD + kg)*GW + c] = wqkv_c[kg*P+p, grpcol(gi)+c]
    # wo_d[p, ko*ODPC + c] = wo_c[ko*P+p, c]
    GW = FLOC // 3
    xt_d = nc.dram_tensor("xt", [P, NCHK * KD * QCH], BF16,
                          kind="ExternalInput").ap()
    wqkv_d = nc.dram_tensor("wqkv", [P, 3 * KD * GW], BF16,
                            kind="ExternalInput").ap()
    wo_d = nc.dram_tensor("wo", [P, KD * ODPC], BF16,
                          kind="ExternalInput").ap()
    sc_d = nc.dram_tensor("sincos2", [P, 2 * S], BF16, kind="ExternalInput").ap()
    mask_d = nc.dram_tensor("maskblk", [n_mask * P, KT], BF16,
                            kind="ExternalInput").ap()
    out_d = nc.dram_tensor("out", [ODPC, TOK], F32, kind="ExternalOutput").ap()

    with tile.TileContext(nc) as tc:
        with tc.tile_pool(name="persist", bufs=1) as persist, \
             tc.tile_pool(name="dram", bufs=1, space="DRAM") as dram:
            Kt = persist.tile([P, TOK], BF16)          # rope'd K^T, all tokens
            v_kd = persist.tile([P, NB * NKT, HD], BF16)
            wo_sb = persist.tile([P, KD, ODPC], BF16)  # preloaded during A
            mkall = persist.tile([P, n_mask, KT], BF16)
            nc.gpsimd.dma_start(
                mkall[:],
                mask_d[:].rearrange("(mb p) q -> p mb q", p=P))
            ident = persist.tile([P, P], BF16)
            nc.gpsimd.memset(ident[:], 0.0)
            nc.gpsimd.affine_select(
                out=ident[:], in_=ident[:],
                compare_op=mybir.AluOpType.not_equal, fill=1.0, base=0,
                pattern=[[-1, P]], channel_multiplier=1)
            # indicator columns for per-head denominator matmuls; head h's
            # denominator lands on the 32-aligned PSUM row 32h so the
            # per-head reciprocal/broadcast can address it directly
            ecol2 = persist.tile([P, HPC, P], BF16)
            nc.vector.memset(ecol2[:], 0.0)
            for h in range(HPC):
                nc.vector.memset(ecol2[:, h, 32 * h:32 * h + 1], 1.0)

            bounce = [dram.tile([QF, QCH], BF16, name=f"bnc{ci}")
                      for ci in range(NCHK)]
            agc = [dram.tile([QF * NCORES, QCH], BF16, name=f"agc{ci}",
                             addr_space="Shared" if NCORES > 4 else "Local")
                   for ci in range(NCHK)]

            with tc.tile_pool(name="phbw", bufs=3) as phbw, \
                 tc.tile_pool(name="phq", bufs=2) as phq, \
                 tc.tile_pool(name="psab", bufs=1, space="PSUM") as psab:
              tails = {}

              # ---------------- attention (generator) -----------------------
              # state[ci] = (d_ps, o_tiles, daccs) shared across head-range
              # parts of one chunk's attention
              astate = {}
              den_ps = {}

              def attn_part(ci, b, qc, qt, h_lo, h_hi, dens_from):
                  """Attention for heads [h_lo, h_hi); emits the denominator
                  matmuls for heads [dens_from, h_hi) (earlier heads' dens
                  can be deferred to a later part to avoid clobbering the
                  previous chunk's denominator bank before its tail runs)."""
                  kts = [kt for kt in range(NKT)
                         if block_cls[(qc, kt)] != 'skip']
                  if ci not in astate:
                      astate[ci] = ({}, {})
                  o_tiles, daccs = astate[ci]

                  def get_dps():
                      if ci not in den_ps:
                          den_ps[ci] = psab.tile(
                              [P, QCH], F32, tag="den", bufs=1,
                              name=f"den{ci}")
                      return den_ps[ci]

                  for h in range(dens_from, h_lo):
                      nc.tensor.matmul(
                          get_dps()[:], ecol2[:, h, :], daccs[h][:],
                          start=(h == 0), stop=False)
                  for h in range(h_lo, h_hi):
                      o_ps = psab.tile([P, QCH], F32, tag="outT", bufs=2,
                                       name=f"o{ci}_{h}")
                      dacc = None
                      for i, kt in enumerate(kts):
                          cls = block_cls[(qc, kt)]
                          qoff = 0
                          if RESTRICT and cls != 'full':
                              qoff = kt * KT - qc * QCH
                          st = psab.tile([P, QCH], F32, tag="st", bufs=2,
                                         name=f"st{ci}_{h}_{i}")
                          nc.tensor.matmul(
                              st[:, qoff:],
                              Kt[:, b * S + kt * KT:b * S + (kt + 1) * KT],
                              qt[:, h, qoff:],
                              start=True, stop=True)
                          yield
                          pt = phbw.tile([P, QCH], BF16, tag="pt", bufs=6,
                                         name=f"pt{ci}_{h}_{i}")
                          nc.scalar.activation(
                              pt[:, qoff:], st[:, qoff:],
                              mybir.ActivationFunctionType.Exp,
                              bias=0.0, scale=float(SCALE))
                          if cls != 'full':
                              nc.vector.tensor_mul(
                                  pt[:, qoff:qoff + KT],
                                  pt[:, qoff:qoff + KT], mkall[:, cls, :])
                          first, last = (i == 0), (i == len(kts) - 1)
                          nc.tensor.matmul(
                              o_ps[:, qoff:], v_kd[:, b * NKT + kt, :],
                              pt[:, qoff:], start=first, stop=last)
                          yield
                          if i == 0:
                              dacc = pt
                          elif i == 1:
                              dsum = phbw.tile([P, QCH], BF16, tag="dsum",
                                               bufs=4, name=f"ds{ci}_{h}")
                              nc.vector.tensor_copy(dsum[:], dacc[:])
                              nc.vector.tensor_add(
                                  dsum[:, qoff:], dsum[:, qoff:],
                                  pt[:, qoff:])
                              dacc = dsum
                          else:
                              nc.vector.tensor_add(
                                  dacc[:, qoff:], dacc[:, qoff:],
                                  pt[:, qoff:])
                      daccs[h] = dacc
                      if h >= dens_from:
                          yield
                          nc.tensor.matmul(
                              get_dps()[:], ecol2[:, h, :], dacc[:],
                              start=(h == 0), stop=(h == HPC - 1))
                      o_sb = phbw.tile([P, QCH], BF16, tag="osbuf", bufs=8,
                                       name=f"ou{ci}_{h}")
                      nc.vector.tensor_copy(o_sb[:], o_ps[:])
                      o_tiles[h] = o_sb

                  if h_hi < HPC:
                      return
                  d_ps = den_ps[ci]

                  def tail():
                      for h in range(HPC):
                          # denominators sit in 32-aligned rows of d_ps, so
                          # the per-head reciprocal + partition_broadcast
                          # stay on legal partition bases (no PE broadcast
                          # matmul needed)
                          dnh = phbw.tile([1, QCH], F32, tag="dnh", bufs=4,
                                          name=f"dnh{ci}_{h}")
                          nc.vector.tensor_copy(
                              dnh[:], d_ps[32 * h:32 * h + 1, :])
                          ivf = phbw.tile([1, QCH], F32, tag="ivf", bufs=4,
                                          name=f"ivf{ci}_{h}")
                          nc.vector.reciprocal_approx_fast(ivf[:], dnh[:])
                          ivh = phbw.tile([1, QCH], BF16, tag="ivh", bufs=4,
                                          name=f"ivh{ci}_{h}")
                          nc.vector.tensor_copy(ivh[:], ivf[:])
                          ib = phbw.tile([P, QCH], BF16, tag="ibc", bufs=2,
                                         name=f"ib{ci}_{h}")
                          nc.gpsimd.partition_broadcast(ib[:], ivh[:])
                          at = phbw.tile([P, QCH], BF16, tag="at", bufs=2,
                                         name=f"at{ci}_{h}")
                          nc.vector.tensor_mul(at[:], o_tiles[h][:], ib[:])
                          nc.sync.dma_start(
                              bounce[ci][h * P:(h + 1) * P, :], at[:])
                      nc.gpsimd.collective_compute(
                          "AllGather", mybir.AluOpType.bypass,
                          replica_groups=[list(range(NCORES))],
                          ins=[bounce[ci].opt()], outs=[agc[ci].opt()])
                      astate.pop(ci)
                      den_ps.pop(ci)
                  tails[ci] = tail

              def attn_gen(ci, b, qc, qt):
                  return attn_part(ci, b, qc, qt, 0, HPC, 0)

              # ---------------- projection + RoPE (generator) ---------------
              with tc.tile_pool(name="pha", bufs=1) as pha, \
                   tc.tile_pool(name="phax", bufs=2) as phax, \
                   tc.tile_pool(name="phat", bufs=2) as phat, \
                   tc.tile_pool(name="psA", bufs=1, space="PSUM") as psA:
                wq_sb = pha.tile([P, KD, FLOC], BF16)
                sc_sb = pha.tile([P, 2 * S], BF16)
                cosT = sc_sb[:, 0:S]
                sinT = sc_sb[:, S:2 * S]

                def load_xt_half(ch, half):
                    xt_sb = phax.tile([P, KH, QCH], BF16, tag="xt",
                                      name=f"xt{ch}_{half}")
                    c0 = (ch * KD + half * KH) * QCH
                    nc.sync.dma_start(
                        xt_sb[:], xt_d[:, c0:c0 + KH * QCH]
                        .rearrange("p (ko t) -> p ko t", t=QCH))
                    return xt_sb

                m_groups = [[4, 5], [0, 1], [2, 3]]
                # startup: dma_start costs ~0.85us of ISSUE time on the
                # issuing engine's queue, so batch wq loads 4 k-tiles per
                # DMA and spread the streams across queues: x on sync, wq
                # column-group 0 (K/V, consumed first) on scalar, group 1
                # on vector, group 2 on gpsimd -- all in consumption order
                xt0 = {}
                for half in range(2):
                    xt0[half] = phax.tile([P, KH, QCH], BF16, tag="xt",
                                          name=f"xt0_{half}")
                qk = KH // 4
                cgrps = [(g[0] * P, (g[-1] + 1) * P) for g in m_groups]
                weng = [nc.scalar, nc.gpsimd, nc.sync]
                for q4 in range(8):
                    half, qh = q4 // 4, q4 % 4
                    x0 = q4 * qk * QCH
                    nc.sync.dma_start(
                        xt0[half][:, qh * qk:(qh + 1) * qk, :],
                        xt_d[:, x0:x0 + qk * QCH]
                        .rearrange("p (ko t) -> p ko t", t=QCH))
                    for gi in range(2):
                        c0, c1 = cgrps[gi]
                        k0 = q4 * 4
                        w0 = (gi * KD + k0) * GW
                        weng[gi].dma_start(
                            wq_sb[:, k0:k0 + 4, c0:c1],
                            wqkv_d[:, w0:w0 + 4 * GW]
                            .rearrange("p (ko c) -> p ko c", c=GW))
                c0, c1 = cgrps[2]
                for q4 in range(8):
                    k0 = q4 * 4
                    w0 = (2 * KD + k0) * GW
                    nc.sync.dma_start(
                        wq_sb[:, k0:k0 + 4, c0:c1],
                        wqkv_d[:, w0:w0 + 4 * GW]
                        .rearrange("p (ko c) -> p ko c", c=GW))
                xt0_h0, xt0_h1 = xt0[0], xt0[1]
                nc.scalar.dma_start(sc_sb[:], sc_d[:])

                def proj_gen(ci, b, cb, first=False):
                    col0 = ci * QCH
                    s0 = col0 % S
                    if first:
                        halves = [xt0_h0, xt0_h1]
                    else:
                        halves = [load_xt_half(ci, 0), load_xt_half(ci, 1)]
                    qt = phq.tile([P, HPC, QCH], BF16, tag="qt",
                                  name=f"qt{ci}")
                    vt = phq.tile([P, QCH], BF16, tag="vt", name=f"vt{ci}")

                    for gi, grp in enumerate(m_groups):
                        pss = {m: psA.tile([P, QCH], F32, tag="pa", bufs=3,
                                           name=f"pa{ci}_{m}")
                               for m in grp}
                        for half in range(2):
                            xt_sb = halves[half]
                            for k in range(KH):
                                kg = half * KH + k
                                for m in grp:
                                    nc.tensor.matmul(
                                        pss[m][:],
                                        wq_sb[:, kg, m * P:(m + 1) * P],
                                        xt_sb[:, k, :],
                                        start=(kg == 0), stop=(kg == KD - 1))
                                    yield
                        for m in grp:
                            if m == MT - 1:        # V: no rope
                                nc.vector.tensor_copy(vt[:], pss[m][:])
                                continue
                            dst = (qt[:, m, :] if m < HPC
                                   else Kt[:, col0:col0 + QCH])
                            t1 = phat.tile([P, QCH], F32, tag="t1",
                                           name=f"t1_{ci}_{m}")
                            t2 = phat.tile([P, QCH], F32, tag="t2",
                                           name=f"t2_{ci}_{m}")
                            nc.vector.tensor_mul(t1[:], pss[m][:],
                                                 cosT[:, s0:s0 + QCH])
                            nc.vector.tensor_mul(t2[0:HHD, :],
                                                 pss[m][HHD:P, :],
                                                 sinT[0:HHD, s0:s0 + QCH])
                            nc.vector.tensor_mul(t2[HHD:P, :],
                                                 pss[m][0:HHD, :],
                                                 sinT[HHD:P, s0:s0 + QCH])
                            nc.vector.tensor_add(dst, t1[:], t2[:])
                        if gi == 1:
                            # V^T -> V via PE transposes; the V^T copy ran
                            # during group 1's matmuls, so no PE wait here
                            for j in range(KTC):
                                pt_ps = psab.tile([P, P], BF16, tag="st",
                                                  bufs=2, name=f"vt{ci}_{j}")
                                nc.tensor.transpose(
                                    pt_ps[:], vt[:, j * KT:(j + 1) * KT],
                                    ident[:])
                                yield
                                nc.vector.tensor_copy(
                                    v_kd[:, b * NKT + cb * KTC + j, :],
                                    pt_ps[:])
                    qts[ci] = qt

                # ---------------- interleaved emission ------------------
                def interleave(main, filler, f):
                    """Pull f filler items per main item; return filler."""
                    acc = 0.0
                    for _ in main:
                        if filler is None:
                            continue
                        acc += f
                        while acc >= 1.0:
                            acc -= 1.0
                            if next(filler, _SENT) is _SENT:
                                filler = None
                                break
                    return filler

                _SENT = object()
                qts = {}

                def drain(gen):
                    for _ in gen:
                        pass

                chunks = [(b * NQC + qc, b, qc)
                          for b in range(NB) for qc in range(NQC)]

                def n_attn_items(qc):
                    kts = [kt for kt in range(NKT)
                           if block_cls[(qc, kt)] != 'skip']
                    return HPC * (2 * len(kts) + 1)

                # slot 0: proj(c0) solid
                drain(proj_gen(0, chunks[0][1], chunks[0][2], first=True))
                def chain(*mk_gens):
                    for mk in mk_gens:
                        yield from mk()

                # slots 1..NCHK-1: attn(c_{i-1}) x proj(c_i); the LAST slot
                # also runs the first half (heads 0-1) of the final chunk's
                # attention as extra filler so the slot boundary and the
                # final AllGather chain shorten
                for i in range(1, NCHK):
                    ci, b, qc = chunks[i]
                    pci, pb, pqc = chunks[i - 1]
                    if i >= 2:
                        tails.pop(chunks[i - 2][0])()
                    if i == NCHK - 1 and HEADSPLIT:
                        pg = chain(
                            lambda: proj_gen(ci, b, qc),
                            lambda: attn_part(ci, b, qc, qts[ci],
                                              0, HPC // 2, HPC))
                        f = 2.0
                    else:
                        pg = proj_gen(ci, b, qc)
                        f = 200.0 / n_attn_items(pqc)
                    ag = attn_gen(pci, pb, pqc, qts[pci])
                    rest = interleave(ag, pg, f)
                    if rest is not None:
                        drain(rest)
                    if i == 2:
                        # wo preload on the now-idle gpsimd queue, batched;
                        # deferred past startup so it doesn't steal HBM
                        # bandwidth from the first chunk's loads
                        for k4 in range(0, KD, 4):
                            nc.gpsimd.dma_start(
                                wo_sb[:, k4:k4 + 4, :],
                                wo_d[:, k4 * ODPC:(k4 + 4) * ODPC]
                                .rearrange("p (ko c) -> p ko c", c=ODPC))

              # ------------- phase C: out^T = wo_c^T @ attn^T -------------
              # chunk PAIRS at N=1024 (rhs free dim 1024 halves the matmul
              # instruction count); interleaved with the last chunk's attn
              with tc.tile_pool(name="phcx", bufs=3) as phcx, \
                   tc.tile_pool(name="phco", bufs=2) as phco, \
                   tc.tile_pool(name="psC", bufs=1, space="PSUM") as psC:
                def c_gen():
                    kh2 = KD // 2
                    for ci in range(NCHK):
                        tok0 = ci * QCH
                        ahs = []
                        for half in range(2):
                            agt = phcx.tile([P, kh2, QCH], BF16, tag="agt",
                                            name=f"agt{ci}_{half}")
                            nc.sync.dma_start(
                                agt[:],
                                agc[ci][half * kh2 * P:
                                        (half + 1) * kh2 * P, :]
                                .rearrange("(ko p) t -> p ko t", p=P))
                            ahs.append(agt)
                        for md in range(ODPC // P):
                            po = psC.tile([P, QCH], F32, tag="po",
                                          bufs=3, name=f"po{ci}_{md}")
                            for kf in range(KD):
                                nc.tensor.matmul(
                                    po[:],
                                    wo_sb[:, kf, md * P:(md + 1) * P],
                                    ahs[kf // kh2][:, kf % kh2, :],
                                    start=(kf == 0), stop=(kf == KD - 1))
                                yield
                            osb = phco.tile([P, QCH], F32, tag="osb",
                                            name=f"osb{ci}_{md}")
                            nc.vector.tensor_copy(osb[:], po[:])
                            nc.sync.dma_start(
                                out_d[md * P:(md + 1) * P,
                                      tok0:tok0 + QCH], osb[:])

                lci, lb, lqc = chunks[-1]
                tails.pop(chunks[-2][0])()
                cg = c_gen()
                # second half (heads 2-3) of the final chunk's attention,
                # plus the deferred denominator matmuls of heads 0-1
                if HEADSPLIT:
                    ag = attn_part(lci, lb, lqc, qts[lci], HPC // 2, HPC, 0)
                else:
                    ag = attn_gen(lci, lb, lqc, qts[lci])
                # low filler ratio: just enough C matmuls to cover exp
                # latency, so attn c7 (and its AllGather) finishes early
                # and the gather hides under the remaining C work
                rest = interleave(ag, cg, 2.0)
                tails.pop(lci)()
                if rest is not None:
                    drain(rest)

    nc.compile()
    return nc


def _host_prep(x, wqkv, wo, sincos, full_causal_mask, start_pos,
               NB, S, D, HPC, NCORES):
    """Shard, cast, and lay out inputs; classify mask blocks."""
    bf16 = ml_dtypes.bfloat16
    TOK = NB * S
    H = HPC * NCORES
    QF = HPC * HD
    NQC = S // QCH
    NKT = S // KT
    ODPC = D // NCORES
    q_sz = H * HD

    KD = D // P
    NCHK = TOK // QCH
    # per-partition-contiguous slab: xt[p, ci, ko, t] = x[ci*QCH+t, ko*P+p]
    xt = np.ascontiguousarray(
        x.reshape(NCHK, QCH, KD, P).transpose(3, 0, 2, 1)
    ).astype(bf16).reshape(P, NCHK * KD * QCH)

    # effective mask: [q, k] (batch-shared), incl. the cache-validity term
    m_eff = np.asarray(full_causal_mask[0, 0], dtype=bool)
    m_eff = m_eff[start_pos:start_pos + S, :S].copy()
    valid = np.arange(S) < (start_pos + S)
    m_eff &= valid[None, :]

    block_cls = {}
    mixed_blocks = []
    qc_mask = {}
    for qc in range(NQC):
        first = len(mixed_blocks)
        for kt in range(NKT):
            blk = m_eff[qc * QCH:(qc + 1) * QCH, kt * KT:(kt + 1) * KT]
            if blk.all():
                block_cls[(qc, kt)] = 'full'
            elif not blk.any():
                block_cls[(qc, kt)] = 'skip'
            else:
                # mixed block must be diagonal: all-zero below the valid
                # q-range, all-one above the triangle subblock
                qoff = kt * KT - qc * QCH
                assert 0 <= qoff < QCH, (qc, kt)
                blkT = blk.T  # [k, q]
                assert not blkT[:, :qoff].any()
                assert blkT[:, qoff + KT:].all()
                block_cls[(qc, kt)] = len(mixed_blocks)
                mixed_blocks.append(np.ascontiguousarray(
                    blkT[:, qoff:qoff + KT].astype(np.float32)))
        cnt = len(mixed_blocks) - first
        if cnt:
            qc_mask[qc] = (first, cnt)
    n_mixed = len(mixed_blocks)
    if n_mixed:
        maskblk = np.concatenate(mixed_blocks, axis=0).astype(bf16)
    else:
        maskblk = np.zeros((P, KT), dtype=bf16)

    # rope tables, transposed + duplicated halves; sin rows 0:64 negated
    sc = np.asarray(sincos[start_pos:start_pos + S], dtype=np.float32)
    sin, cos = sc[:, :HHD], sc[:, HHD:]
    cosT2 = np.concatenate([cos.T, cos.T], axis=0)           # [128, S]
    sinT2 = np.concatenate([-sin.T, sin.T], axis=0)          # [128, S]
    sincos2 = np.concatenate([cosT2, sinT2], axis=1).astype(bf16)

    in_maps = []
    for c in range(NCORES):
        qcols = np.asarray(wqkv[:, c * QF:(c + 1) * QF])
        kcols = np.asarray(wqkv[:, q_sz + c * HD:q_sz + (c + 1) * HD])
        vcols = np.asarray(
            wqkv[:, q_sz + NCORES * HD + c * HD:
                 q_sz + NCORES * HD + (c + 1) * HD])
        wqkv_c = np.concatenate([qcols, kcols, vcols], axis=1)
        # regroup columns in kernel consumption order (K/V group first),
        # then swizzle: wqs[p, gi, kg, c] = wq_g[kg*P+p, gi*GW+c]
        GW = (QF + 2 * HD) // 3
        wq_g = np.concatenate([wqkv_c[:, 2 * GW:], wqkv_c[:, :GW],
                               wqkv_c[:, GW:2 * GW]], axis=1)
        wqkv_c = np.ascontiguousarray(
            wq_g.reshape(KD, P, 3, GW).transpose(1, 2, 0, 3)
        ).astype(bf16).reshape(P, 3 * KD * GW)
        wo_c = np.ascontiguousarray(
            np.asarray(wo[:, c * ODPC:(c + 1) * ODPC])
            .reshape(KD, P, ODPC).transpose(1, 0, 2)
        ).astype(bf16).reshape(P, KD * ODPC)
        in_maps.append({
            "xt": xt, "wqkv": wqkv_c, "wo": wo_c,
            "sincos2": sincos2, "maskblk": maskblk,
        })
    return in_maps, block_cls, n_mixed, qc_mask


_CACHE = {}


def run_distributed(x, wqkv, wo, sincos, full_causal_mask, start_pos,
                    NB, S, D, HPC, NCORES, trace=False, tmpdir=None):
    in_maps, block_cls, n_mixed, qc_mask = _host_prep(
        x, wqkv, wo, sincos, full_causal_mask, start_pos,
        NB, S, D, HPC, NCORES)
    key = (NB, S, D, HPC, NCORES,
           tuple(sorted((k, v) for k, v in block_cls.items())))
    if key not in _CACHE:
        _CACHE[key] = build_graph(NB, S, D, HPC, NCORES, block_cls, n_mixed,
                                  qc_mask)
    nc = _CACHE[key]
    res = run_bass_kernel_spmd(nc, in_maps, list(range(NCORES)), trace=trace,
                               tmpdir=tmpdir)
    TOK = NB * S
    out = np.empty((TOK, D), dtype=np.float32)
    ODPC = D // NCORES
    for c in range(NCORES):
        out[:, c * ODPC:(c + 1) * ODPC] = res.results[c]["out"].T
    return out.reshape(NB, S, D), res


def kernel(x, wqkv, wo, sincos, cache_k, cache_v, full_causal_mask,
           start_pos) -> np.ndarray:
    x = np.asarray(x)
    start_pos = int(np.asarray(start_pos))
    B, S_, D_ = x.shape
    assert start_pos == 0, "prefill-only kernel (seq fills the whole cache)"
    out, _ = run_distributed(
        x, np.asarray(wqkv), np.asarray(wo), np.asarray(sincos),
        np.asarray(full_causal_mask), start_pos,
        NB=B, S=S_, D=D_, HPC=4, NCORES=8)
    return out
